# revision 18
# baseline (speedup 1.0000x reference)
"""DLRM dot-interaction kernel for Trainium2 (Bass/Tile), 8-core data parallel.

Computes, for each sample b:
    combined = concat([dense[b], sparse[b]])          # [27, 128]
    C = combined @ combined.T                          # [27, 27] gram
    out[b] = concat([dense[b], triu_flat(C)])          # [506]

Device strategy (per core, S = 4096 samples):
  - Host pre-transposes inputs to X^T layout [D=128, S, 27] so the
    contraction dim D sits on SBUF partitions (no on-device transpose).
  - QUAD-PACKED matmuls: 4 consecutive samples form one matmul with
    lhsT = rhs = [128, 108] (their X^T blocks side by side), producing a
    [108, 108] PSUM tile whose 4 diagonal 27x27 blocks are the grams.
    This keeps the PE instruction stream small (1024 matmuls/core) so
    sequencer instruction fetch from HBM stays ahead of execution --
    per-sample matmuls (4096+ LdW/MM pairs) stall ~3.2us on every 16KB
    IRAM block fetch, which dominated earlier versions.
  - One PSUM bank holds 4 quads (4 x 108 fp32 <= 512); full-bank
    evictions to SBUF alternate between DVE and ACT.
  - Output DMAs ship, per chunk, the 4 diagonal-block classes
    (sample index mod 4) as strided writes to DRAM laid out [27, S, 27]
    (gram row-major), making the host-side triu gather 27 contiguous
    slice copies. Cross-sample garbage blocks are evicted to SBUF but
    never DMA'd.
  - Input loads alternate across both HWDGE rings; output stores use
    SWDGE so their slow HBM write receipts don't stall the input stream.
  - Dense passthrough (output cols 0:128) is assembled on the host.
"""

import os
import sys

import numpy as np

for _p in (
    "/root/.axon_site",
    "/root/.axon_site/_ro/trn_rl_repo",
    "/opt/trn_rl_repo",
):
    if os.path.isdir(_p) and _p not in sys.path:
        sys.path.append(_p)

import concourse.bacc as bacc
import concourse.bass as bass
import concourse.mybir as mybir
import concourse.tile as tile

NF = 27  # combined features (1 dense + 26 sparse)
D = 128  # embedding dim
B = 32768  # batch
NCORES = 8
S = B // NCORES  # samples per core

F32 = mybir.dt.float32
QW = 4 * NF  # quad width: 108 columns = 4 samples

# Upper-triangle (incl. diagonal) flattened offsets: row n starts at TOFF[n],
# length 27 - n. Matches np.triu_indices(27) row-major order.
TOFF = np.concatenate([[0], np.cumsum(NF - np.arange(NF))]).astype(np.int64)
NPAIRS = int(TOFF[NF])  # 378
DOUT = D + NPAIRS  # 506


def build_nc(s_per_core=S, kb=32, ib=4, qb=4):
    """Build the per-core Bass program.

    kb: PSUM banks per chunk (chunk = kb * qb * 4 samples)
    ib: banks per input-DMA tile
    qb: quads per PSUM bank (qb * 108 fp32 <= 512)
    """
    bank_sz = 4 * qb  # samples per PSUM bank
    c_sz = kb * bank_sz  # samples per chunk
    assert s_per_core % c_sz == 0, (s_per_core, c_sz)
    assert kb % ib == 0
    nchunks = s_per_core // c_sz

    # Bacc (not raw Bass): its compile() pass legalizes multi-wait matmuls
    # (raw Bass emits >1 wait on LdWeights, which walrus codegen rejects).
    nc = bacc.Bacc("TRN2", target_bir_lowering=False, debug=False)
    xt = nc.dram_tensor("xt", [D, s_per_core * NF], F32, kind="ExternalInput")
    gram = nc.dram_tensor("gram", [NF, s_per_core, NF], F32, kind="ExternalOutput")

    with tile.TileContext(nc) as tc:
        with (
            tc.tile_pool(name="xin", bufs=6) as xin_pool,
            tc.tile_pool(name="gbuf", bufs=2) as gbuf_pool,
            tc.tile_pool(name="ps", bufs=8, space="PSUM") as ps_pool,
        ):
            in_engines = [nc.sync, nc.scalar]
            evict_engines = [nc.vector, nc.vector, nc.scalar]  # 2:1 DVE:ACT
            rr = {"in": 0, "ev": 0}

            for c0 in range(nchunks):
                gbuf = gbuf_pool.tile([128, kb * qb * QW], F32)
                for bi in range(kb // ib):
                    s_base = c0 * c_sz + bi * ib * bank_sz
                    xin = xin_pool.tile([D, ib * bank_sz * NF], F32)
                    eng = in_engines[rr["in"] % 2]
                    rr["in"] += 1
                    eng.dma_start(
                        out=xin[:],
                        in_=xt[:, s_base * NF : (s_base + ib * bank_sz) * NF],
                    )
                    for bh in range(ib):
                        b = bi * ib + bh
                        ps = ps_pool.tile([128, qb * QW], F32)
                        for q in range(qb):
                            loc = (bh * bank_sz + q * 4) * NF
                            nc.tensor.matmul(
                                ps[0:QW, q * QW : (q + 1) * QW],
                                xin[:, loc : loc + QW],
                                xin[:, loc : loc + QW],
                                start=True,
                                stop=True,
                            )
                        dst = gbuf[0:QW, b * qb * QW : (b + 1) * qb * QW]
                        if rr["ev"] % 3 < 2:
                            nc.vector.tensor_copy(dst, ps[0:QW, :])
                        else:
                            nc.scalar.copy(dst, ps[0:QW, :])
                        rr["ev"] += 1
                # Output: one DMA per diagonal-block class i (= sample mod 4).
                # Sample (c0, b, q, i) has global index c0*c_sz + b*bank_sz +
                # q*4 + i and lives in gbuf at partitions 27i..27i+26, free
                # offset b*qb*QW + q*QW + 27i.
                src_all = gbuf[:].rearrange(
                    "p (b q w) -> p b q w", b=kb, q=qb
                )
                dst_all = gram[:, c0 * c_sz : (c0 + 1) * c_sz, :].rearrange(
                    "p (b q four) m -> p b q four m", b=kb, q=qb
                )
                for i in range(4):
                    nc.gpsimd.dma_start(
                        out=dst_all[:, :, :, i],
                        in_=src_all[
                            27 * i : 27 * i + NF, :, :, 27 * i : 27 * i + NF
                        ],
                    )
    nc.finalize()  # runs Bacc.compile() (reg alloc, wait legalization)
    return nc


def host_pack_inputs(dense_features, sparse_features):
    """[B,128] + [B,26,128] -> X^T layout [128, B, 27] fp32."""
    bsz = dense_features.shape[0]
    xt = np.empty((D, bsz, NF), dtype=np.float32)
    xt[:, :, 0] = np.asarray(dense_features, dtype=np.float32).T
    xt[:, :, 1:] = np.asarray(sparse_features, dtype=np.float32).transpose(2, 0, 1)
    return xt


def host_core_input(xt, c, s_per_core=S):
    """Slice core c's shard as [128, S*27]."""
    return np.ascontiguousarray(
        xt[:, c * s_per_core : (c + 1) * s_per_core, :]
    ).reshape(D, s_per_core * NF)


def host_unpack_output(dense_features, gram_t):
    """dense [B,128] + gram_t [27, B, 27] -> [B, 506] (dense ++ triu)."""
    bsz = dense_features.shape[0]
    out = np.empty((bsz, DOUT), dtype=np.float32)
    out[:, :D] = dense_features
    for n in range(NF):
        lo = D + int(TOFF[n])
        out[:, lo : lo + NF - n] = gram_t[n, :, n:]
    return out


_NC_CACHE = {}


def _get_nc():
    key = (S,)
    if key not in _NC_CACHE:
        _NC_CACHE[key] = build_nc(S)
    return _NC_CACHE[key]


def kernel(dense_features, sparse_features):
    from concourse.bass_utils import run_bass_kernel_spmd

    dense_features = np.asarray(dense_features, dtype=np.float32)
    sparse_features = np.asarray(sparse_features, dtype=np.float32)
    xt = host_pack_inputs(dense_features, sparse_features)

    in_maps = [{"xt": host_core_input(xt, c)} for c in range(NCORES)]
    nc = _get_nc()
    res = run_bass_kernel_spmd(nc, in_maps, core_ids=list(range(NCORES)))
    gram_t = np.concatenate([r["gram"] for r in res.results], axis=1)  # [27, B, 27]
    return host_unpack_output(dense_features, gram_t)
